# revision 21
# baseline (speedup 1.0000x reference)
"""CrossAttentionFusion kernel for 8 Trainium2 NeuronCores.

Math (per reference): two seq-len-1 cross-attention blocks (each reduces to
out_proj(v_proj(x)) = one fused E x E matmul), residual+LN after each, then a
4E FFN with exact-erf GELU and a final residual+LN.

Strategy:
  - Pure data parallel over the batch (16384 rows -> 2048 rows per core).
  - Feature-major ("transposed") activations on device: tiles are
    [128 features, batch] so every matmul is lhsT(=W.T chunk).T @ x.T with no
    on-device transposes. LayerNorm reductions over features run on the PE
    (ones-vector matmuls).
  - Attention matmuls in fp8 e4m3 with DoubleRow perf mode (2 contraction
    subtiles per pass = 2x bf16 rate). Weights are host-scaled by 2048 so the
    fp8 mantissa is used well; the residual+bias (host-folded, also x2048,
    bf16) keeps the whole pre-LN tensor in the scaled domain, and LN is
    scale-invariant, so no descale op is ever needed.
  - FFN matmuls in bf16 (fp8 there would blow the 2e-2 error budget).
    wf1/wf2 fully SBUF-resident; c and h spill through DRAM in bf16.
  - Attention pairs are fused on the host: W1 = w_out1 @ wv1 etc (exact
    algebra, seq_len==1). b1/b2 folded into the host residuals.
"""

import os
import sys

import numpy as np

sys.path.insert(0, "/opt/trn_rl_repo")

E = 1024
B = 16384
NCORES = 8
R = B // NCORES          # rows per core
CH = E // 128            # feature chunks (8)
F = 4 * E                # ffn hidden (4096)
FCH = F // 128           # ffn hidden chunks (32)
N = 512                  # batch tile
NT = R // N              # 4
WS = 2048.0              # fp8 attention weight scale

# CoreSim does not implement Gelu; tests may set KERNEL_GELU=Tanh for
# structural sim checks. Hardware always uses the real (erf) Gelu.
_GELU_FUNC = os.environ.get("KERNEL_GELU", "Gelu")

_RUNNER = None


def _emit_program(nc, repeats=1, phases="acd"):
    import concourse.bass as bass
    import concourse.mybir as mybir
    import concourse.tile as tile

    F32 = mybir.dt.float32
    F32R = mybir.dt.float32r
    BF16 = mybir.dt.bfloat16
    FP8 = mybir.dt.float8e4
    AF = mybir.ActivationFunctionType
    OP = mybir.AluOpType
    DR = mybir.MatmulPerfMode.DoubleRow
    ts = bass.ts

    # activations are tile-major [128, NT, CH, N] flattened so every DMA is
    # contiguous per partition (host prepares this layout)
    xq = nc.declare_dram_parameter("xq", [128, NT * CH * N], FP8, isOutput=False)
    res1 = nc.declare_dram_parameter("res1", [128, NT * CH * N], BF16, isOutput=False)
    res2 = nc.declare_dram_parameter("res2", [128, NT * CH * N], BF16, isOutput=False)
    w1q = nc.declare_dram_parameter("w1q", [E, E], FP8, isOutput=False)
    w2q = nc.declare_dram_parameter("w2q", [E, E], FP8, isOutput=False)
    wf1t = nc.declare_dram_parameter("wf1t", [E, F], BF16, isOutput=False)
    wf2t = nc.declare_dram_parameter("wf2t", [F, E], BF16, isOutput=False)
    # packed per-partition params: [128, c] with [p, c] = v[c*128+p]
    bf1 = nc.declare_dram_parameter("bf1", [128, FCH], F32, isOutput=False)
    bf2 = nc.declare_dram_parameter("bf2", [128, CH], F32, isOutput=False)
    # ln params: 6 groups of CH cols: g1 be1 g2 be2 g3 be3
    lnp = nc.declare_dram_parameter("lnp", [128, 6 * CH], F32, isOutput=False)
    ones_in = nc.declare_dram_parameter("ones_in", [128, 1], F32R, isOutput=False)
    ones1_in = nc.declare_dram_parameter("ones1_in", [1, 128], F32R, isOutput=False)
    ot = nc.declare_dram_parameter("ot", [E, R], F32R, isOutput=True)

    xqr = xq.rearrange("p (n c j) -> p n c j", n=NT, c=CH)
    r1r = res1.rearrange("p (n c j) -> p n c j", n=NT, c=CH)
    r2r = res2.rearrange("p (n c j) -> p n c j", n=NT, c=CH)
    otr = ot.rearrange("(c p) r -> p c r", p=128)
    w1r = w1q.rearrange("(c p) m -> p c m", p=128)
    w2r = w2q.rearrange("(c p) m -> p c m", p=128)
    wf1r = wf1t.rearrange("(k p) m -> p k m", p=128)
    wf2r = wf2t.rearrange("(k p) m -> p k m", p=128)

    with nc.allow_low_precision("fp8/bf16 matmul pipeline; f32 psum accum"), \
         tile.TileContext(nc) as tc:
        from contextlib import ExitStack

        with tc.tile_pool(name="dram", bufs=1, space="DRAM") as dram, \
             tc.tile_pool(name="const", bufs=1) as const:
            cbuf = dram.tile([128, NT, CH, N], BF16)
            hbuf = dram.tile([128, NT, FCH, N], BF16)

            bf1sb = const.tile([128, FCH], F32)
            bf2sb = const.tile([128, CH], F32)
            lnsb = const.tile([128, 6 * CH], F32)
            ones128 = const.tile([128, 1], F32R)
            ones1 = const.tile([1, 128], F32R)
            epsb = const.tile([1, 1], F32)
            nc.gpsimd.dma_start(out=bf1sb[:], in_=bf1[:])
            nc.gpsimd.dma_start(out=bf2sb[:], in_=bf2[:])
            nc.gpsimd.dma_start(out=lnsb[:], in_=lnp[:])
            nc.gpsimd.dma_start(out=ones128[:], in_=ones_in[:])
            nc.gpsimd.dma_start(out=ones1[:], in_=ones1_in[:])
            nc.vector.memset(epsb[:], 1e-5)

            def layer_norm(ctx_pools, r_t, width, ln_idx, outs):
                """LN over features of r_t [128, CH, width] f32r (destroyed).

                outs: list of tiles to receive g*x_norm + be (any dtype).
                Stats and broadcasts on PE; squares/apply on DVE (f32 SBUF
                2x mode); g/be apply + psum->SBUF broadcast evacs on ACT.
                """
                sqp, stp, bcp, ps_st, ps_bc = ctx_pools
                g_col = lnsb[:, 2 * ln_idx * CH: (2 * ln_idx + 1) * CH]
                be_col = lnsb[:, (2 * ln_idx + 1) * CH: (2 * ln_idx + 2) * CH]
                s_ps = ps_st.tile([1, width], F32, tag="s_ps")
                q_ps = ps_st.tile([1, width], F32, tag="q_ps")
                for m in range(CH):
                    nc.tensor.matmul(s_ps[:], ones128[:], r_t[:, m, :],
                                     start=(m == 0), stop=(m == CH - 1))
                for m in range(CH):
                    sq = sqp.tile([128, width], F32R, tag="sq")
                    nc.vector.tensor_tensor(out=sq[:], in0=r_t[:, m, :],
                                            in1=r_t[:, m, :], op=OP.mult)
                    nc.tensor.matmul(q_ps[:], ones128[:], sq[:],
                                     start=(m == 0), stop=(m == CH - 1))
                mu_t = stp.tile([1, width], F32R, tag="mu")
                mu2_t = stp.tile([1, width], F32, tag="mu2")
                var_t = stp.tile([1, width], F32, tag="var")
                rstd_t = stp.tile([1, width], F32R, tag="rstd")
                nc.vector.tensor_scalar(out=mu_t[:], in0=s_ps[:],
                                        scalar1=1.0 / E, scalar2=None,
                                        op0=OP.mult)
                nc.vector.tensor_tensor(out=mu2_t[:], in0=mu_t[:], in1=mu_t[:],
                                        op=OP.mult)
                nc.vector.scalar_tensor_tensor(out=var_t[:], in0=q_ps[:],
                                               scalar=1.0 / E, in1=mu2_t[:],
                                               op0=OP.mult, op1=OP.subtract)
                nc.scalar.activation(out=var_t[:], in_=var_t[:], func=AF.Sqrt,
                                     bias=epsb[:])
                nc.vector.reciprocal(out=rstd_t[:], in_=var_t[:])
                mu_ps = ps_bc.tile([128, width], F32, tag="mu_ps")
                rstd_ps = ps_bc.tile([128, width], F32, tag="rstd_ps")
                nc.tensor.matmul(mu_ps[:], ones1[:], mu_t[:], start=True, stop=True)
                nc.tensor.matmul(rstd_ps[:], ones1[:], rstd_t[:], start=True, stop=True)
                mu_sb = bcp.tile([128, width], F32R, tag="mu_sb")
                rstd_sb = bcp.tile([128, width], F32R, tag="rstd_sb")
                nc.scalar.activation(out=mu_sb[:], in_=mu_ps[:], func=AF.Identity)
                nc.scalar.activation(out=rstd_sb[:], in_=rstd_ps[:], func=AF.Identity)
                for m in range(CH):
                    nc.vector.tensor_tensor(out=r_t[:, m, :], in0=r_t[:, m, :],
                                            in1=mu_sb[:], op=OP.subtract)
                    nc.vector.tensor_tensor(out=r_t[:, m, :], in0=r_t[:, m, :],
                                            in1=rstd_sb[:], op=OP.mult)
                    for out_t in outs:
                        nc.scalar.activation(out=out_t[:, m, :], in_=r_t[:, m, :],
                                             func=AF.Identity,
                                             scale=g_col[:, m:m + 1],
                                             bias=be_col[:, m:m + 1])

            for rep in range(repeats):
                # ffn1 weights live through phases A+C; the DMA issues before
                # phase A so the load overlaps it instead of stalling the
                # A->C boundary. Freed before phase D (SBUF pressure there).
                acs = ExitStack()
                pw1 = acs.enter_context(tc.tile_pool(name="pw1", bufs=1))
                wf1sb = pw1.tile([128, CH, F], BF16)
                for piece in range(4):
                    pk = slice(piece * 2, piece * 2 + 2)
                    nc.sync.dma_start(out=wf1sb[:, pk, :], in_=wf1r[:, pk, :])

                # ------------ Phase A: att1+LN1+att2+LN2 -> c ------------
                if "a" in phases:
                  with ExitStack() as ab:
                      wab = ab.enter_context(tc.tile_pool(name="wab", bufs=1))
                      px = ab.enter_context(tc.tile_pool(name="px", bufs=2))
                      prs = ab.enter_context(tc.tile_pool(name="prs", bufs=2))
                      pr = ab.enter_context(tc.tile_pool(name="pr", bufs=2))
                      pimg = ab.enter_context(tc.tile_pool(name="pimg", bufs=2))
                      pimq = ab.enter_context(tc.tile_pool(name="pimq", bufs=1))
                      pc = ab.enter_context(tc.tile_pool(name="pc", bufs=2))
                      sqp = ab.enter_context(tc.tile_pool(name="sqp", bufs=3))
                      stp = ab.enter_context(tc.tile_pool(name="stp", bufs=1))
                      bcp = ab.enter_context(tc.tile_pool(name="bcp", bufs=1))
                      psA = ab.enter_context(tc.tile_pool(name="psA", bufs=4, space="PSUM"))
                      ps_st = ab.enter_context(tc.tile_pool(name="ps_st", bufs=1, space="PSUM"))
                      ps_bc = ab.enter_context(tc.tile_pool(name="ps_bc", bufs=1, space="PSUM"))
                      lnpools = (sqp, stp, bcp, ps_st, ps_bc)

                      w1sb = wab.tile([128, CH, E], FP8)
                      w2sb = wab.tile([128, CH, E], FP8)
                      HCH = CH // 2
                      nc.sync.dma_start(out=w1sb[:, :HCH, :], in_=w1r[:, :HCH, :])
                      nc.sync.dma_start(out=w1sb[:, HCH:, :], in_=w1r[:, HCH:, :])
                      nc.sync.dma_start(out=w2sb[:, :HCH, :], in_=w2r[:, :HCH, :])
                      nc.sync.dma_start(out=w2sb[:, HCH:, :], in_=w2r[:, HCH:, :])

                      def attention(wsb, rhs_tile, res_tile, out_r):
                          """out_r[m] = DR-fp8 (wsb.T @ rhs)[m] + res[m] (f32r)."""
                          for mg in range(2):
                              accs = []
                              for _mi in range(4):
                                  acc_g = psA.tile([128, N], F32, tag="acc",
                                                   name=f"acc_g{_mi}")
                                  accs.append(acc_g)
                              for kp in range(CH // 2):
                                  for mi in range(4):
                                      m = mg * 4 + mi
                                      nc.tensor.matmul(
                                          accs[mi][:],
                                          wsb[:, 2 * kp:2 * kp + 2, ts(m, 128)],
                                          rhs_tile[:, 2 * kp:2 * kp + 2, :],
                                          start=(kp == 0), stop=(kp == CH // 2 - 1),
                                          perf_mode=DR)
                              for mi in range(4):
                                  m = mg * 4 + mi
                                  nc.vector.tensor_tensor(
                                      out=out_r[:, m, :], in0=accs[mi][:],
                                      in1=res_tile[:, m, :], op=OP.add)

                      for n in range(NT):
                          xq_t = px.tile([128, CH, N], FP8, tag="xq_t")
                          nc.sync.dma_start(out=xq_t[:], in_=xqr[:, n])
                          rs1_t = prs.tile([128, CH, N], BF16, tag="rs1")
                          rs2_t = prs.tile([128, CH, N], BF16, tag="rs2")
                          nc.sync.dma_start(out=rs1_t[:], in_=r1r[:, n])
                          nc.sync.dma_start(out=rs2_t[:], in_=r2r[:, n])

                          r1 = pr.tile([128, CH, N], F32R, tag="r")
                          attention(w1sb, xq_t, rs1_t, r1)
                          img = pimg.tile([128, CH, N], BF16, tag="img")
                          imq = pimq.tile([128, CH, N], FP8, tag="imq")
                          layer_norm(lnpools, r1, N, 0, [img, imq])

                          r2 = pr.tile([128, CH, N], F32R, tag="r")
                          attention(w2sb, imq, rs2_t, r2)
                          ct = pc.tile([128, CH, N], BF16, tag="ct")
                          layer_norm(lnpools, r2, N, 1, [r2])
                          for m in range(CH):
                              nc.gpsimd.tensor_tensor(
                                  out=ct[:, m, :], in0=r2[:, m, :],
                                  in1=img[:, m, :], op=OP.add)
                          nc.sync.dma_start(out=cbuf[:, n], in_=ct[:])

                # ------------ Phase C: h = gelu(wf1 @ c + bf1) ------------
                if "c" in phases:
                  with ExitStack() as pcx:
                      pcc = pcx.enter_context(tc.tile_pool(name="pcc", bufs=2))
                      ph = pcx.enter_context(tc.tile_pool(name="ph", bufs=2))
                      psC = pcx.enter_context(tc.tile_pool(name="psC", bufs=6, space="PSUM"))

                      for n in range(NT):
                          ct = pcc.tile([128, CH, N], BF16, tag="ct")
                          nc.sync.dma_start(out=ct[:], in_=cbuf[:, n])
                          hst = ph.tile([128, FCH, N], BF16, tag="hst")
                          for mj in range(FCH):
                              acc = psC.tile([128, N], F32, tag="accC")
                              for k in range(CH):
                                  nc.tensor.matmul(acc[:], wf1sb[:, k, ts(mj, 128)],
                                                   ct[:, k, :],
                                                   start=(k == 0), stop=(k == CH - 1))
                              nc.scalar.activation(out=hst[:, mj, :], in_=acc[:],
                                                   func=getattr(AF, _GELU_FUNC),
                                                   bias=bf1sb[:, mj:mj + 1])
                          for piece in range(4):
                              pk = slice(piece * CH, (piece + 1) * CH)
                              nc.sync.dma_start(out=hbuf[:, n, pk, :],
                                                in_=hst[:, pk, :])

                acs.close()

                # ------------ Phase D: ffn2 + residual + LN3 ------------
                if "d" in phases:
                  with ExitStack() as pdx:
                      pw2 = pdx.enter_context(tc.tile_pool(name="pw2", bufs=1))
                      phD = pdx.enter_context(tc.tile_pool(name="phD", bufs=2))
                      pcD = pdx.enter_context(tc.tile_pool(name="pcD", bufs=2))
                      pr3 = pdx.enter_context(tc.tile_pool(name="pr3", bufs=2))
                      sqpD = pdx.enter_context(tc.tile_pool(name="sqpD", bufs=3))
                      stpD = pdx.enter_context(tc.tile_pool(name="stpD", bufs=1))
                      bcpD = pdx.enter_context(tc.tile_pool(name="bcpD", bufs=2))
                      psD = pdx.enter_context(tc.tile_pool(name="psD", bufs=4, space="PSUM"))
                      ps_stD = pdx.enter_context(tc.tile_pool(name="ps_stD", bufs=1, space="PSUM"))
                      ps_bcD = pdx.enter_context(tc.tile_pool(name="ps_bcD", bufs=1, space="PSUM"))
                      lnpoolsD = (sqpD, stpD, bcpD, ps_stD, ps_bcD)

                      wf2sb = pw2.tile([128, FCH, E], BF16)
                      for piece in range(4):
                          pk = slice(piece * CH, (piece + 1) * CH)
                          nc.sync.dma_start(out=wf2sb[:, pk, :], in_=wf2r[:, pk, :])
                      for n in range(NT):
                          sl = slice(n * N, (n + 1) * N)
                          ht = phD.tile([128, FCH, N], BF16, tag="ht")
                          for piece in range(4):
                              pk = slice(piece * CH, (piece + 1) * CH)
                              nc.sync.dma_start(out=ht[:, pk, :],
                                                in_=hbuf[:, n, pk, :])
                          ct = pcD.tile([128, CH, N], BF16, tag="ct")
                          nc.sync.dma_start(out=ct[:], in_=cbuf[:, n])
                          r3 = pr3.tile([128, CH, N], F32R, tag="r3")
                          for m in range(CH):
                              acc = psD.tile([128, N], F32, tag="accD")
                              for k in range(FCH):
                                  nc.tensor.matmul(acc[:], wf2sb[:, k, ts(m, 128)],
                                                   ht[:, k, :],
                                                   start=(k == 0), stop=(k == FCH - 1))
                              nc.vector.scalar_tensor_tensor(
                                  out=r3[:, m, :], in0=acc[:],
                                  scalar=bf2sb[:, m:m + 1],
                                  in1=ct[:, m, :], op0=OP.add, op1=OP.add)
                          layer_norm(lnpoolsD, r3, N, 2, [r3])
                          nc.sync.dma_start(out=otr[:, :, sl], in_=r3[:])

    nc.finalize()
    return nc


def _build(repeats=1):
    from concourse import bacc

    nc = bacc.Bacc()
    return _emit_program(nc, repeats=repeats)


def _make_exec(nc, n_cores=NCORES):
    """Cached jitted SPMD executor, mirroring run_bass_via_pjrt's multi-core
    branch so repeated calls reuse the compiled NEFF."""
    import jax
    import concourse.mybir as mybir
    from concourse import bass2jax
    from jax.experimental.shard_map import shard_map
    from jax.sharding import Mesh, PartitionSpec

    bass2jax.install_neuronx_cc_hook()

    partition_name = nc.partition_id_tensor.name if nc.partition_id_tensor else None
    in_names, out_names, out_avals, zero_shapes = [], [], [], []
    for alloc in nc.m.functions[0].allocations:
        if not isinstance(alloc, mybir.MemoryLocationSet):
            continue
        name = alloc.memorylocations[0].name
        if alloc.kind == "ExternalInput":
            if name != partition_name:
                in_names.append(name)
        elif alloc.kind == "ExternalOutput":
            out_names.append(name)
            shape = tuple(alloc.tensor_shape)
            dtype = mybir.dt.np(alloc.dtype)
            out_avals.append(jax.core.ShapedArray(shape, dtype))
            zero_shapes.append((shape, dtype))
    n_params = len(in_names)
    n_outs = len(out_names)
    all_names = in_names + out_names
    if partition_name is not None:
        all_names = all_names + [partition_name]

    def _body(*args):
        operands = list(args)
        if partition_name is not None:
            operands.append(bass2jax.partition_id_tensor())
        outs = bass2jax._bass_exec_p.bind(
            *operands,
            out_avals=tuple(out_avals),
            in_names=tuple(all_names),
            out_names=tuple(out_names),
            lowering_input_output_aliases=(),
            sim_require_finite=True,
            sim_require_nnan=True,
            nc=nc,
        )
        return tuple(outs)

    devices = jax.devices()[:n_cores]
    mesh = Mesh(np.asarray(devices), ("core",))
    sharded_names = set(in_names)
    in_specs = (PartitionSpec("core"),) * (n_params + n_outs)
    out_specs = (PartitionSpec("core"),) * n_outs
    donate = tuple(range(n_params, n_params + n_outs))
    sharded = jax.jit(
        shard_map(_body, mesh=mesh, in_specs=in_specs, out_specs=out_specs,
                  check_rep=False),
        donate_argnums=donate, keep_unused=True)

    def run(in_maps):
        concat_in = [
            np.concatenate([np.asarray(in_maps[c][nm]) for c in range(n_cores)], axis=0)
            if nm in sharded_names else np.asarray(in_maps[0][nm])
            for nm in in_names
        ]
        concat_zeros = [
            np.zeros((n_cores * s[0],) + tuple(s[1:]), dt) for (s, dt) in zero_shapes
        ]
        out_arrs = sharded(*concat_in, *concat_zeros)
        out_arrs = [np.asarray(a) for a in out_arrs]
        return [
            {nm: out_arrs[i].reshape(n_cores, *out_avals[i].shape)[c]
             for i, nm in enumerate(out_names)}
            for c in range(n_cores)
        ]

    run.sharded_names = sharded_names
    run.in_names = in_names
    run.out_names = out_names
    run.sharded = sharded
    run.n_cores = n_cores
    run.out_avals = out_avals
    run.zero_shapes = zero_shapes
    run.body = _body
    run.mesh = mesh
    run.in_specs = in_specs
    run.out_specs = out_specs
    run.nc = nc
    return run


def _pack_pp(v, ch):
    """bias vector [ch*128] -> per-partition [128, ch]."""
    return np.ascontiguousarray(v.reshape(ch, 128).T.astype(np.float32))


def prepare_in_maps(img_feat, txt_feat, w_in1, b_in1, w_out1, b_out1,
                    w_in2, b_in2, w_out2, b_out2,
                    g1, be1, g2, be2, g3, be3,
                    w_ffn1, b_ffn1, w_ffn2, b_ffn2):
    import ml_dtypes
    f32 = np.float32
    bf16 = ml_dtypes.bfloat16
    e4 = ml_dtypes.float8_e4m3
    img = np.asarray(img_feat, f32)
    txt = np.asarray(txt_feat, f32)
    w_in1 = np.asarray(w_in1, f32); b_in1 = np.asarray(b_in1, f32)
    w_out1 = np.asarray(w_out1, f32); b_out1 = np.asarray(b_out1, f32)
    w_in2 = np.asarray(w_in2, f32); b_in2 = np.asarray(b_in2, f32)
    w_out2 = np.asarray(w_out2, f32); b_out2 = np.asarray(b_out2, f32)
    w_ffn1 = np.asarray(w_ffn1, f32); b_ffn1 = np.asarray(b_ffn1, f32)
    w_ffn2 = np.asarray(w_ffn2, f32); b_ffn2 = np.asarray(b_ffn2, f32)

    wv1 = w_in1[2 * E:]
    bv1 = b_in1[2 * E:]
    W1 = w_out1 @ wv1                      # att1 == txt @ W1.T + b1
    b1 = w_out1 @ bv1 + b_out1
    wv2 = w_in2[2 * E:]
    bv2 = b_in2[2 * E:]
    W2 = w_out2 @ wv2
    b2 = w_out2 @ bv2 + b_out2

    def q8t(w):
        # scaled fp8 weights, transposed to [k, m] layout
        return np.ascontiguousarray(
            np.clip(w.T * WS, -240.0, 240.0).astype(e4))

    lnp = np.concatenate([
        _pack_pp(np.asarray(v, f32), CH)
        for v in (g1, be1, g2, be2, g3, be3)], axis=1)

    shared = {
        "w1q": q8t(W1),
        "w2q": q8t(W2),
        "wf1t": np.ascontiguousarray(w_ffn1.T.astype(bf16)),
        "wf2t": np.ascontiguousarray(w_ffn2.T.astype(bf16)),
        "bf1": _pack_pp(b_ffn1, FCH),
        "bf2": _pack_pp(b_ffn2, CH),
        "lnp": lnp,
        "ones_in": np.ones((128, 1), f32),
        "ones1_in": np.ones((1, 128), f32),
    }
    res1_full = (WS * (img + b1[None, :])).astype(bf16)
    res2_full = (WS * (txt + b2[None, :])).astype(bf16)
    xq_full = np.clip(txt, -240.0, 240.0).astype(e4)

    def tilemajor(a):
        # per-core slice [R, E] -> [128, NT*CH*N] with
        # out[p, n, c, j] = a[n*N+j, c*128+p]
        t = a.reshape(NT, N, CH, 128).transpose(3, 0, 2, 1)
        return np.ascontiguousarray(t.reshape(128, NT * CH * N))

    in_maps = []
    for c in range(NCORES):
        sh = slice(c * R, (c + 1) * R)
        m = dict(shared)
        m["xq"] = tilemajor(xq_full[sh])
        m["res1"] = tilemajor(res1_full[sh])
        m["res2"] = tilemajor(res2_full[sh])
        in_maps.append(m)
    return in_maps


def get_runner():
    global _RUNNER
    if _RUNNER is None:
        nc = _build()
        _RUNNER = _make_exec(nc)
    return _RUNNER


def kernel(**inputs) -> np.ndarray:
    run = get_runner()
    in_maps = prepare_in_maps(**inputs)
    results = run(in_maps)
    out = np.empty((B, E), np.float32)
    for c in range(NCORES):
        out[c * R:(c + 1) * R] = results[c]["ot"].T
    return out
